# revision 16
# baseline (speedup 1.0000x reference)
"""Gaussian falloff vortex-velocity kernel for Trainium2 (8 NeuronCores).

Math: out[b,h,w,:] = sum_n tau_n * exp(-r2/sig_n^2) / sqrt(r2) * (d2, -d1)
with d1 = py - y_n, d2 = px - x_n, r2 = d1^2 + d2^2.

Device algorithm (per core, H sharded 8 ways). Let v = a'*(r2+eps) with
a' = 1/sig^2, so the falloff g = exp(-v)/sqrt(v) (per-particle factors
q = exp(a'*eps)*sqrt(a') are folded into the S-weights).

Per half-tile (128 particles x 1024 points), X = -v comes from a K=31
triple-bf16-split PE contraction (negated, telescoped row order).  The
exponent w = -v - 0.5*ln(v) is then built by one of three paths chosen
per half to balance engine load:
  I: DVE fast-log ts  w1 = -c*float(int32bits(X)) + K  (+-1.5% sawtooth,
     centered), then PE identity-matmul accumulates w1 onto X.
  A: ACT Ln(-X) -> lt; DVE stt w = -0.5*lt + X  (exact).
  B: DVE fast-log ts -> w1; DVE stt w = X + w1.
One ACT Exp then yields g = exp(-v)/sqrt(v) in fp16.  PE S-matmuls
compute S_r(p) = sum_n w_rn*g_np for r in {tau, tau*x, tau*y} with
single-fp16 weights, accumulated into 32-aligned partition slots (4
point-tiles per PSUM bank via tile_position).  S tiles are copied
PSUM->SBUF and DMAed out; the host computes u = px*S0 - S1,
v = S2 - py*S0 and assembles the [B,H,W,2] output.

Ln and Exp share one ACT table set (natural_log_exp_and_others).
"""

import sys

import numpy as np

B, H, W, N = 2, 256, 256, 512
NCORES = 8
HPC = H // NCORES          # 32 rows per core
PPB = HPC * W              # 8192 points per batch per core
NT = PPB // 512            # 16 point-tiles of 512 per batch
NK = N // 128              # 4 particle blocks
KROWS = 31
EPS0, EPS1 = 2e-6, 1.5e-6

LN2 = float(np.log(2.0))
CFAST = 0.5 * LN2 * 2.0**-23             # fast-log slope
KBIAS = 0.5 * LN2 * (127.0 - 0.0430357)  # sawtooth-centered constant
K1 = -CFAST * (2.0**31) + KBIAS          # ts constant (sign bit of -v absorbed)

# Path pattern per half-tile: I (PE merge), A (ACT Ln), B (DVE merge).
PATHS = "IIIA"  # 3 I : 1 A — keeps PE dense (full p-state)

_cache = {}


def _bass_modules():
    if "/opt/trn_rl_repo" not in sys.path:
        sys.path.insert(0, "/opt/trn_rl_repo")
    import concourse.bass as bass
    import concourse.mybir as mybir
    import concourse.tile as tile
    from concourse import bacc
    from concourse.bass_utils import run_bass_kernel_spmd

    return bass, mybir, tile, run_bass_kernel_spmd, bacc


def _pin_act_table_set():
    """Make the table-load pass satisfy Ln/Exp only from the combined set so
    alternating Ln/Exp instructions never thrash ACT table loads."""
    import concourse.bacc as bacc_mod
    import concourse.mybir as mybir

    if getattr(bacc_mod, "_act_tables_pinned", False):
        return
    orig = bacc_mod.get_activation_tables
    ln_exp = {mybir.ActivationFunctionType.Ln, mybir.ActivationFunctionType.Exp}

    def patched(arch):
        tables = orig(arch)
        keep = "natural_log_exp_and_others"
        if keep not in tables:
            return tables
        return {
            name: (funcs if name == keep else (funcs - ln_exp))
            for name, funcs in tables.items()
        }

    bacc_mod.get_activation_tables = patched
    bacc_mod._act_tables_pinned = True


def _build_nc(step_ms=0.00146):
    bass, mybir, tile, _, bacc = _bass_modules()
    _pin_act_table_set()
    f32 = mybir.dt.float32
    i32 = mybir.dt.int32
    bf16 = mybir.dt.bfloat16
    fp16 = mybir.dt.float16
    AF = mybir.ActivationFunctionType
    ALU = mybir.AluOpType

    nc = bacc.Bacc(None)
    rhs_d = nc.declare_dram_parameter("rhs", [B, 16, KROWS, PPB // 16], bf16, isOutput=False)
    lhst_d = nc.declare_dram_parameter("lhst", [B, KROWS, N], bf16, isOutput=False)
    wq_d = nc.declare_dram_parameter("wq", [128, B * NK * 3], fp16, isOutput=False)
    eye_d = nc.declare_dram_parameter("eye", [128, 128], fp16, isOutput=False)
    out_d = nc.declare_dram_parameter("out", [B, NT // 4, 4, 3, 512], f32, isOutput=True)

    NHALF = B * NT * 2  # 64 half-tiles; half h covers particle blocks 2h, 2h+1

    def path_of(u):
        return PATHS[u % len(PATHS)]

    with tile.TileContext(nc) as tc:
        with (
            tc.tile_pool(name="const", bufs=1) as cpool,
            tc.tile_pool(name="w1p", bufs=3) as w1pool,
            tc.tile_pool(name="ltp", bufs=3) as ltpool,
            tc.tile_pool(name="gp", bufs=3) as gpool,
            tc.tile_pool(name="stg", bufs=2) as stgpool,
            tc.tile_pool(name="xp", bufs=3, space=bass.MemorySpace.PSUM) as xpool,
            tc.tile_pool(name="sap", bufs=2, space=bass.MemorySpace.PSUM) as spool,
        ):
            rhs_sb, lhs_sb = [], []
            for b in range(B):
                t = cpool.tile([KROWS, PPB], bf16, tag=f"rhs{b}")
                rhs_sb.append(t)
                t2 = cpool.tile([KROWS, N], bf16, tag=f"lhs{b}")
                lhs_sb.append(t2)
            CW = PPB // 16
            # spread input DMAs across SP / DVE / ACT hwdge queues
            nc.sync.dma_start(rhs_sb[0][:, 0:CW], rhs_d[0, 0])
            nc.sync.dma_start(lhs_sb[0][:], lhst_d[0])
            wq = cpool.tile([128, B * NK * 3], fp16, tag="wq")
            nc.sync.dma_start(wq[:], wq_d[:])
            eye = cpool.tile([128, 128], fp16, tag="eye")
            nc.sync.dma_start(eye[:], eye_d[:])
            engs = [nc.sync, nc.sync]
            for c in range(1, 16):
                engs[c % 2].dma_start(rhs_sb[0][:, c * CW : (c + 1) * CW], rhs_d[0, c])
            nc.sync.dma_start(lhs_sb[1][:], lhst_d[1])
            for c in range(16):
                engs[c % 2].dma_start(rhs_sb[1][:, c * CW : (c + 1) * CW], rhs_d[1, c])

            X_t, w_t, g_t, sacc_t = {}, {}, {}, {}

            def half_info(u):
                gi, h = divmod(u, 2)
                b, T = divmod(gi, NT)
                return b, T, h

            def stage1(u):  # v-matmuls -> X = -v
                b, T, h = half_info(u)
                merge = path_of(u) == "I"
                X = xpool.tile([128, 1024], f32, tag="X", name=f"X{u}")
                X_t[u] = X
                sl = slice(T * 512, (T + 1) * 512)
                for hh in range(2):
                    k = 2 * h + hh
                    nc.tensor.matmul(
                        X[:, hh * 512 : (hh + 1) * 512],
                        lhs_sb[b][:, k * 128 : (k + 1) * 128],
                        rhs_sb[b][:, sl],
                        start=True,
                        stop=not merge,
                    )

            def stage2(u):  # build exponent w (I: PE merge, A: Ln+stt, B: DVE)
                X = X_t[u]
                p = path_of(u)
                if p == "A":
                    lt = ltpool.tile([128, 1024], f32, tag="lt")
                    nc.scalar.activation(lt[:], X[:], AF.Ln, scale=-1.0)
                    w = ltpool.tile([128, 1024], f32, tag="w")
                    nc.vector.scalar_tensor_tensor(
                        w[:], lt[:], -0.5, X[:], ALU.mult, ALU.add
                    )
                    w_t[u] = w
                elif p == "B":
                    w1 = w1pool.tile([128, 1024], fp16, tag="w1")
                    nc.vector.tensor_scalar(
                        w1[:], X[:].bitcast(i32), -CFAST, K1, ALU.mult, ALU.add
                    )
                    w = ltpool.tile([128, 1024], f32, tag="wb")
                    nc.vector.scalar_tensor_tensor(
                        w[:], X[:], 1.0, w1[:], ALU.mult, ALU.add
                    )
                    w_t[u] = w
                else:
                    w1 = w1pool.tile([128, 1024], fp16, tag="w1")
                    nc.vector.tensor_scalar(
                        w1[:], X[:].bitcast(i32), -CFAST, K1, ALU.mult, ALU.add
                    )
                    for hh in range(2):
                        nc.tensor.matmul(
                            X[:, hh * 512 : (hh + 1) * 512],
                            eye[:],
                            w1[:, hh * 512 : (hh + 1) * 512],
                            start=False,
                            stop=True,
                        )

            def stage3(u):  # Exp -> g, then S-matmuls
                b, T, h = half_info(u)
                p = path_of(u)
                g = gpool.tile([128, 1024], fp16, tag="g")
                g_t[u] = g
                if p == "I":
                    nc.scalar.activation(g[:], X_t[u][:], AF.Exp)
                else:
                    nc.scalar.activation(g[:], w_t.pop(u)[:], AF.Exp)
                X_t.pop(u, None)
                q, s = divmod(T, 4)
                key = (b, q)
                if key not in sacc_t:
                    sacc_t[key] = spool.tile([128, 512], f32, tag="sacc", name=f"sa{b}_{q}")
                sacc = sacc_t[key]
                base = 32 * s
                for hh in range(2):
                    k = 2 * h + hh
                    c3 = (b * NK + k) * 3
                    nc.tensor.matmul(
                        sacc[base : base + 3, :],
                        wq[:, c3 : c3 + 3],
                        g[:, hh * 512 : (hh + 1) * 512],
                        start=(k == 0),
                        stop=(k == NK - 1),
                        tile_position=(0, base),
                        skip_group_check=(base != 0),
                    )
                if h == 1 and s == 3:
                    finish_q(b, q)

            def finish_q(b, q):
                sacc = sacc_t.pop((b, q))
                stage = stgpool.tile([128, 512], f32, tag="stage", name=f"st{b}_{q}")
                nc.vector.tensor_copy(stage[:], sacc[:])
                for s in range(4):
                    nc.sync.dma_start(out_d[b, q, s], stage[32 * s : 32 * s + 3, :])

            STEP = step_ms
            for u in range(NHALF + 2):
                t_it = STEP * u
                if u < NHALF:
                    with tc.tile_wait_until(t_it):
                        stage1(u)
                if 1 <= u <= NHALF:
                    with tc.tile_wait_until(t_it + STEP * 0.33):
                        stage2(u - 1)
                if 2 <= u <= NHALF + 1:
                    with tc.tile_wait_until(t_it + STEP * 0.66):
                        stage3(u - 2)
    nc.compile()
    return nc


def _split3(a, bf):
    h = a.astype(bf)
    m = (a - h.astype(np.float64)).astype(bf)
    l = (a - h.astype(np.float64) - m.astype(np.float64)).astype(bf)
    return h, m, l


def _prep_inputs(vortex_feature, points):
    import ml_dtypes

    bf = ml_dtypes.bfloat16
    vf = np.asarray(vortex_feature, dtype=np.float64)
    pts_full = np.asarray(points, dtype=np.float64)
    y, x, tau = vf[:, :, 0], vf[:, :, 1], vf[:, :, 2]
    sig2 = vf[:, :, 3] ** 2
    a_n = 1.0 / sig2                       # a' = 1/sig^2 (v = a'(r2+eps))
    eps_n = EPS0 + EPS1 * (y * y + x * x)

    # lhsT rows [B, KROWS, N]: NEGATED triple-split entries (X = -v).
    lhst = np.zeros((B, KROWS, N), dtype=bf)
    for b in range(B):
        A3 = _split3(-a_n[b], bf)
        CY3 = _split3(2.0 * a_n[b] * y[b], bf)
        CX3 = _split3(2.0 * a_n[b] * x[b], bf)
        AYY3 = _split3(-a_n[b] * y[b] * y[b], bf)
        AXX3 = _split3(-a_n[b] * x[b] * x[b], bf)
        aeps = (-a_n[b] * eps_n[b]).astype(bf)
        rows = []
        for (uh, um, ul) in (A3, CY3):
            rows += [uh, uh, um, uh, ul, um]
        rows += list(AYY3)
        for (uh, um, ul) in (A3, CX3):
            rows += [uh, uh, um, uh, ul, um]
        rows += list(AXX3)
        rows.append(aeps)
        lhst[b] = np.stack(rows, 0)

    # fp16 S-weights: rows {tau*q, tau*x*q, tau*y*q}, q = exp(a'*eps)*sqrt(a')
    q = np.exp(a_n * eps_n) * np.sqrt(a_n)
    wfull = np.stack([tau * q, tau * x * q, tau * y * q], axis=-1)  # [B, N, 3]
    assert np.abs(wfull).max() < 6.0e4, "fp16 S-weight overflow"
    w3 = wfull.astype(np.float16)
    wq = np.ascontiguousarray(
        w3.reshape(B, NK, 128, 3).transpose(2, 0, 1, 3).reshape(128, B * NK * 3)
    )

    eyem = np.eye(128, dtype=np.float16)

    in_maps = []
    for i in range(NCORES):
        slp = pts_full[:, i * HPC : (i + 1) * HPC].reshape(B, PPB, 2)
        pts = np.ascontiguousarray(slp.transpose(0, 2, 1))  # [B, 2, PPB]
        rhs = np.zeros((B, KROWS, PPB), dtype=bf)
        for b in range(B):
            py, px = pts[b, 0], pts[b, 1]
            PYY3 = _split3(py * py, bf)
            PY3 = _split3(py, bf)
            PXX3 = _split3(px * px, bf)
            PX3 = _split3(px, bf)
            ones = np.ones(PPB, dtype=bf)
            rows = []
            for (wh_, wm_, wl_) in (PYY3, PY3):
                rows += [wh_, wm_, wh_, wl_, wh_, wm_]
            rows += [ones] * 3
            for (wh_, wm_, wl_) in (PXX3, PX3):
                rows += [wh_, wm_, wh_, wl_, wh_, wm_]
            rows += [ones] * 3
            rows.append(ones)
            rhs[b] = np.stack(rows, 0)
        rhs16 = np.ascontiguousarray(
            rhs.reshape(B, KROWS, 16, PPB // 16).transpose(0, 2, 1, 3)
        )
        in_maps.append({"rhs": rhs16, "lhst": lhst, "wq": wq, "eye": eyem})
    return in_maps


def _assemble(results, points):
    pts_full = np.asarray(points, dtype=np.float64)
    out = np.zeros((B, H, W, 2), dtype=np.float32)
    for i in range(NCORES):
        o = np.asarray(results[i]["out"]).astype(np.float64)  # [B, 4, 4, 3, 512]
        S = o.transpose(0, 3, 1, 2, 4).reshape(B, 3, PPB)     # [B, r, PPB]
        slp = pts_full[:, i * HPC : (i + 1) * HPC].reshape(B, PPB, 2)
        py, px = slp[..., 0], slp[..., 1]
        u = px * S[:, 0] - S[:, 1]
        v = S[:, 2] - py * S[:, 0]
        uv = np.stack([u, v], axis=-1).reshape(B, HPC, W, 2)
        out[:, i * HPC : (i + 1) * HPC] = uv.astype(np.float32)
    return out


def _run(vortex_feature, points, trace=False):
    _, _, _, run_bass_kernel_spmd, _b = _bass_modules()
    if "nc" not in _cache:
        _cache["nc"] = _build_nc()
    in_maps = _prep_inputs(vortex_feature, points)
    res = run_bass_kernel_spmd(
        _cache["nc"], in_maps, list(range(NCORES)), trace=trace
    )
    return _assemble(res.results, points), res


def kernel(vortex_feature, points):
    out, _ = _run(vortex_feature, points, trace=False)
    return out


# revision 17
# speedup vs baseline: 1.1552x; 1.1552x over previous
"""Gaussian falloff vortex-velocity kernel for Trainium2 (8 NeuronCores).

Math: out[b,h,w,:] = sum_n tau_n * exp(-r2/sig_n^2) / sqrt(r2) * (d2, -d1)
with d1 = py - y_n, d2 = px - x_n, r2 = d1^2 + d2^2.

Device algorithm (per core, H sharded 8 ways). Let v = a'*(r2+eps) with
a' = 1/sig^2, so the falloff g = exp(-v)/sqrt(v) (per-particle factors
q = exp(a'*eps)*sqrt(a') are folded into the S-weights).

Per half-tile (128 particles x 1024 points), X = -v comes from a K=31
triple-bf16-split PE contraction (negated, telescoped row order).  The
exponent w = -v - 0.5*ln(v) is then built by one of three paths chosen
per half to balance engine load:
  I: DVE fast-log ts  w1 = -c*float(int32bits(X)) + K  (+-1.5% sawtooth,
     centered), then PE identity-matmul accumulates w1 onto X.
  A: ACT Ln(-X) -> lt; DVE stt w = -0.5*lt + X  (exact).
  B: DVE fast-log ts -> w1; DVE stt w = X + w1.
One ACT Exp then yields g = exp(-v)/sqrt(v) in fp16.  PE S-matmuls
compute S_r(p) = sum_n w_rn*g_np for r in {tau, tau*x, tau*y} with
single-fp16 weights, accumulated into 32-aligned partition slots (4
point-tiles per PSUM bank via tile_position).  S tiles are copied
PSUM->SBUF and DMAed out; the host computes u = px*S0 - S1,
v = S2 - py*S0 and assembles the [B,H,W,2] output.

Ln and Exp share one ACT table set (natural_log_exp_and_others).
"""

import sys

import numpy as np

B, H, W, N = 2, 256, 256, 512
NCORES = 8
HPC = H // NCORES          # 32 rows per core
PPB = HPC * W              # 8192 points per batch per core
NT = PPB // 512            # 16 point-tiles of 512 per batch
NK = N // 128              # 4 particle blocks
KROWS = 31
EPS0, EPS1 = 2e-6, 1.5e-6

LN2 = float(np.log(2.0))
CFAST = 0.5 * LN2 * 2.0**-23             # fast-log slope
KBIAS = 0.5 * LN2 * (127.0 - 0.0430357)  # sawtooth-centered constant
K1 = -CFAST * (2.0**31) + KBIAS          # ts constant (sign bit of -v absorbed)

# Path pattern per half-tile: I (PE merge), A (ACT Ln), B (DVE merge).
PATHS = "IIIA"  # 3 I : 1 A — keeps PE dense (full p-state)

_cache = {}


def _bass_modules():
    if "/opt/trn_rl_repo" not in sys.path:
        sys.path.insert(0, "/opt/trn_rl_repo")
    import concourse.bass as bass
    import concourse.mybir as mybir
    import concourse.tile as tile
    from concourse import bacc
    from concourse.bass_utils import run_bass_kernel_spmd

    return bass, mybir, tile, run_bass_kernel_spmd, bacc


def _pin_act_table_set():
    """Make the table-load pass satisfy Ln/Exp only from the combined set so
    alternating Ln/Exp instructions never thrash ACT table loads."""
    import concourse.bacc as bacc_mod
    import concourse.mybir as mybir

    if getattr(bacc_mod, "_act_tables_pinned", False):
        return
    orig = bacc_mod.get_activation_tables
    ln_exp = {mybir.ActivationFunctionType.Ln, mybir.ActivationFunctionType.Exp}

    def patched(arch):
        tables = orig(arch)
        keep = "natural_log_exp_and_others"
        if keep not in tables:
            return tables
        return {
            name: (funcs if name == keep else (funcs - ln_exp))
            for name, funcs in tables.items()
        }

    bacc_mod.get_activation_tables = patched
    bacc_mod._act_tables_pinned = True


def _build_nc(step_ms=0.00150):
    bass, mybir, tile, _, bacc = _bass_modules()
    _pin_act_table_set()
    f32 = mybir.dt.float32
    i32 = mybir.dt.int32
    bf16 = mybir.dt.bfloat16
    fp16 = mybir.dt.float16
    AF = mybir.ActivationFunctionType
    ALU = mybir.AluOpType

    nc = bacc.Bacc(None)
    rhs_d = nc.declare_dram_parameter("rhs", [B, 16, KROWS, PPB // 16], bf16, isOutput=False)
    lhst_d = nc.declare_dram_parameter("lhst", [B, KROWS, N], bf16, isOutput=False)
    wq_d = nc.declare_dram_parameter("wq", [128, B * NK * 3], fp16, isOutput=False)
    eye_d = nc.declare_dram_parameter("eye", [128, 128], fp16, isOutput=False)
    out_d = nc.declare_dram_parameter("out", [B, NT // 4, 4, 3, 512], f32, isOutput=True)

    NHALF = B * NT * 2  # 64 half-tiles; half h covers particle blocks 2h, 2h+1

    def path_of(u):
        return PATHS[u % len(PATHS)]

    with tile.TileContext(nc) as tc:
        with (
            tc.tile_pool(name="const", bufs=1) as cpool,
            tc.tile_pool(name="w1p", bufs=3) as w1pool,
            tc.tile_pool(name="ltp", bufs=3) as ltpool,
            tc.tile_pool(name="gp", bufs=3) as gpool,
            tc.tile_pool(name="stg", bufs=2) as stgpool,
            tc.tile_pool(name="xp", bufs=3, space=bass.MemorySpace.PSUM) as xpool,
            tc.tile_pool(name="sap", bufs=2, space=bass.MemorySpace.PSUM) as spool,
        ):
            rhs_sb, lhs_sb = [], []
            for b in range(B):
                t = cpool.tile([KROWS, PPB], bf16, tag=f"rhs{b}")
                rhs_sb.append(t)
                t2 = cpool.tile([KROWS, N], bf16, tag=f"lhs{b}")
                lhs_sb.append(t2)
            CW = PPB // 16
            # spread input DMAs across SP / DVE / ACT hwdge queues
            nc.sync.dma_start(rhs_sb[0][:, 0:CW], rhs_d[0, 0])
            nc.sync.dma_start(lhs_sb[0][:], lhst_d[0])
            wq = cpool.tile([128, B * NK * 3], fp16, tag="wq")
            nc.sync.dma_start(wq[:], wq_d[:])
            eye = cpool.tile([128, 128], fp16, tag="eye")
            nc.sync.dma_start(eye[:], eye_d[:])
            engs = [nc.sync, nc.sync]
            for c in range(1, 16):
                engs[c % 2].dma_start(rhs_sb[0][:, c * CW : (c + 1) * CW], rhs_d[0, c])
            nc.sync.dma_start(lhs_sb[1][:], lhst_d[1])
            for c in range(16):
                engs[c % 2].dma_start(rhs_sb[1][:, c * CW : (c + 1) * CW], rhs_d[1, c])

            X_t, w_t, g_t, sacc_t = {}, {}, {}, {}

            def half_info(u):
                gi, h = divmod(u, 2)
                b, T = divmod(gi, NT)
                return b, T, h

            def stage1(u):  # v-matmuls -> X = -v
                b, T, h = half_info(u)
                merge = path_of(u) == "I"
                X = xpool.tile([128, 1024], f32, tag="X", name=f"X{u}")
                X_t[u] = X
                sl = slice(T * 512, (T + 1) * 512)
                for hh in range(2):
                    k = 2 * h + hh
                    nc.tensor.matmul(
                        X[:, hh * 512 : (hh + 1) * 512],
                        lhs_sb[b][:, k * 128 : (k + 1) * 128],
                        rhs_sb[b][:, sl],
                        start=True,
                        stop=not merge,
                    )

            def stage2(u):  # build exponent w (I: PE merge, A: Ln+stt, B: DVE)
                X = X_t[u]
                p = path_of(u)
                if p == "A":
                    lt = ltpool.tile([128, 1024], f32, tag="lt")
                    nc.scalar.activation(lt[:], X[:], AF.Ln, scale=-1.0)
                    w = ltpool.tile([128, 1024], f32, tag="w")
                    nc.vector.scalar_tensor_tensor(
                        w[:], lt[:], -0.5, X[:], ALU.mult, ALU.add
                    )
                    w_t[u] = w
                elif p == "B":
                    w1 = w1pool.tile([128, 1024], fp16, tag="w1")
                    nc.vector.tensor_scalar(
                        w1[:], X[:].bitcast(i32), -CFAST, K1, ALU.mult, ALU.add
                    )
                    w = ltpool.tile([128, 1024], f32, tag="wb")
                    nc.vector.scalar_tensor_tensor(
                        w[:], X[:], 1.0, w1[:], ALU.mult, ALU.add
                    )
                    w_t[u] = w
                else:
                    w1 = w1pool.tile([128, 1024], fp16, tag="w1")
                    nc.vector.tensor_scalar(
                        w1[:], X[:].bitcast(i32), -CFAST, K1, ALU.mult, ALU.add
                    )
                    for hh in range(2):
                        nc.tensor.matmul(
                            X[:, hh * 512 : (hh + 1) * 512],
                            eye[:],
                            w1[:, hh * 512 : (hh + 1) * 512],
                            start=False,
                            stop=True,
                        )

            def stage3(u):  # Exp -> g, then S-matmuls
                b, T, h = half_info(u)
                p = path_of(u)
                g = gpool.tile([128, 1024], fp16, tag="g")
                g_t[u] = g
                if p == "I":
                    nc.scalar.activation(g[:], X_t[u][:], AF.Exp)
                else:
                    nc.scalar.activation(g[:], w_t.pop(u)[:], AF.Exp)
                X_t.pop(u, None)
                q, s = divmod(T, 4)
                key = (b, q)
                if key not in sacc_t:
                    sacc_t[key] = spool.tile([128, 512], f32, tag="sacc", name=f"sa{b}_{q}")
                sacc = sacc_t[key]
                base = 32 * s
                for hh in range(2):
                    k = 2 * h + hh
                    c3 = (b * NK + k) * 3
                    nc.tensor.matmul(
                        sacc[base : base + 3, :],
                        wq[:, c3 : c3 + 3],
                        g[:, hh * 512 : (hh + 1) * 512],
                        start=(k == 0),
                        stop=(k == NK - 1),
                        tile_position=(0, base),
                        skip_group_check=(base != 0),
                    )
                if h == 1 and s == 3:
                    finish_q(b, q)

            def finish_q(b, q):
                sacc = sacc_t.pop((b, q))
                stage = stgpool.tile([128, 512], f32, tag="stage", name=f"st{b}_{q}")
                nc.vector.tensor_copy(stage[:], sacc[:])
                for s in range(4):
                    nc.sync.dma_start(out_d[b, q, s], stage[32 * s : 32 * s + 3, :])

            STEP = step_ms
            for u in range(NHALF + 2):
                t_it = STEP * u
                if u < NHALF:
                    with tc.tile_wait_until(t_it):
                        stage1(u)
                if 1 <= u <= NHALF:
                    with tc.tile_wait_until(t_it + STEP * 0.33):
                        stage2(u - 1)
                if 2 <= u <= NHALF + 1:
                    with tc.tile_wait_until(t_it + STEP * 0.66):
                        stage3(u - 2)
    nc.compile()
    return nc


def _split3(a, bf):
    h = a.astype(bf)
    m = (a - h.astype(np.float64)).astype(bf)
    l = (a - h.astype(np.float64) - m.astype(np.float64)).astype(bf)
    return h, m, l


def _prep_inputs(vortex_feature, points):
    import ml_dtypes

    bf = ml_dtypes.bfloat16
    vf = np.asarray(vortex_feature, dtype=np.float64)
    pts_full = np.asarray(points, dtype=np.float64)
    y, x, tau = vf[:, :, 0], vf[:, :, 1], vf[:, :, 2]
    sig2 = vf[:, :, 3] ** 2
    a_n = 1.0 / sig2                       # a' = 1/sig^2 (v = a'(r2+eps))
    eps_n = EPS0 + EPS1 * (y * y + x * x)

    # lhsT rows [B, KROWS, N]: NEGATED triple-split entries (X = -v).
    lhst = np.zeros((B, KROWS, N), dtype=bf)
    for b in range(B):
        A3 = _split3(-a_n[b], bf)
        CY3 = _split3(2.0 * a_n[b] * y[b], bf)
        CX3 = _split3(2.0 * a_n[b] * x[b], bf)
        AYY3 = _split3(-a_n[b] * y[b] * y[b], bf)
        AXX3 = _split3(-a_n[b] * x[b] * x[b], bf)
        aeps = (-a_n[b] * eps_n[b]).astype(bf)
        rows = []
        for (uh, um, ul) in (A3, CY3):
            rows += [uh, uh, um, uh, ul, um]
        rows += list(AYY3)
        for (uh, um, ul) in (A3, CX3):
            rows += [uh, uh, um, uh, ul, um]
        rows += list(AXX3)
        rows.append(aeps)
        lhst[b] = np.stack(rows, 0)

    # fp16 S-weights: rows {tau*q, tau*x*q, tau*y*q}, q = exp(a'*eps)*sqrt(a')
    q = np.exp(a_n * eps_n) * np.sqrt(a_n)
    wfull = np.stack([tau * q, tau * x * q, tau * y * q], axis=-1)  # [B, N, 3]
    assert np.abs(wfull).max() < 6.0e4, "fp16 S-weight overflow"
    w3 = wfull.astype(np.float16)
    wq = np.ascontiguousarray(
        w3.reshape(B, NK, 128, 3).transpose(2, 0, 1, 3).reshape(128, B * NK * 3)
    )

    eyem = np.eye(128, dtype=np.float16)

    in_maps = []
    for i in range(NCORES):
        slp = pts_full[:, i * HPC : (i + 1) * HPC].reshape(B, PPB, 2)
        pts = np.ascontiguousarray(slp.transpose(0, 2, 1))  # [B, 2, PPB]
        rhs = np.zeros((B, KROWS, PPB), dtype=bf)
        for b in range(B):
            py, px = pts[b, 0], pts[b, 1]
            PYY3 = _split3(py * py, bf)
            PY3 = _split3(py, bf)
            PXX3 = _split3(px * px, bf)
            PX3 = _split3(px, bf)
            ones = np.ones(PPB, dtype=bf)
            rows = []
            for (wh_, wm_, wl_) in (PYY3, PY3):
                rows += [wh_, wm_, wh_, wl_, wh_, wm_]
            rows += [ones] * 3
            for (wh_, wm_, wl_) in (PXX3, PX3):
                rows += [wh_, wm_, wh_, wl_, wh_, wm_]
            rows += [ones] * 3
            rows.append(ones)
            rhs[b] = np.stack(rows, 0)
        rhs16 = np.ascontiguousarray(
            rhs.reshape(B, KROWS, 16, PPB // 16).transpose(0, 2, 1, 3)
        )
        in_maps.append({"rhs": rhs16, "lhst": lhst, "wq": wq, "eye": eyem})
    return in_maps


def _assemble(results, points):
    pts_full = np.asarray(points, dtype=np.float64)
    out = np.zeros((B, H, W, 2), dtype=np.float32)
    for i in range(NCORES):
        o = np.asarray(results[i]["out"]).astype(np.float64)  # [B, 4, 4, 3, 512]
        S = o.transpose(0, 3, 1, 2, 4).reshape(B, 3, PPB)     # [B, r, PPB]
        slp = pts_full[:, i * HPC : (i + 1) * HPC].reshape(B, PPB, 2)
        py, px = slp[..., 0], slp[..., 1]
        u = px * S[:, 0] - S[:, 1]
        v = S[:, 2] - py * S[:, 0]
        uv = np.stack([u, v], axis=-1).reshape(B, HPC, W, 2)
        out[:, i * HPC : (i + 1) * HPC] = uv.astype(np.float32)
    return out


def _run(vortex_feature, points, trace=False):
    _, _, _, run_bass_kernel_spmd, _b = _bass_modules()
    if "nc" not in _cache:
        _cache["nc"] = _build_nc()
    in_maps = _prep_inputs(vortex_feature, points)
    res = run_bass_kernel_spmd(
        _cache["nc"], in_maps, list(range(NCORES)), trace=trace
    )
    return _assemble(res.results, points), res


def kernel(vortex_feature, points):
    out, _ = _run(vortex_feature, points, trace=False)
    return out
